# revision 65
# baseline (speedup 1.0000x reference)
"""Multi-head attention (B=2, S=2048, D=1024, H=16, HD=64) on 8 TRN2 cores.

Sharding (hybrid DP/TP, SPMD one-graph):
  core c: batch b = c//4, head-group g = c%4 (heads 4g..4g+3 of batch b).
  - single x input per core: xqT = x^T with token quarters XOR-permuted
    (local quarter s = true quarter s^g).  Q/K/V all project from it;
    key/value token order is irrelevant to attention, and the query
    permutation makes the exchange SPMD-uniform.
  - QKV projections Megatron column-split (4 heads per core), bf16.
  - attention per (j2=head-pair, sq=quarter): QK^T row-packed
    2 heads/pass; exp split ~5/8 ACT (exact) / 3/8 DVE (Schraudolph
    int16-bitcast bf16); PV bf16 with a ones column producing the
    softmax denominator in row 64.
  - softmax divide: reciprocal off PSUM row 64, GPSIMD
    partition_broadcast to 64 rows, one DVE multiply.
  - exchange: per round m in {1,2,3}: both j2 planes staged to DRAM,
    ONE pairwise (2-rank) AllGather per m (cores g <-> g^m).  Rounds
    ordered (1,2,3,0-local) so all gathers overlap the remaining
    compute.
  - O-projection token-sharded: per output tile, 6 gathered chunks
    (sel-blended, 2 DVE ops) + 2 local chunks (hd-stacked, K=128),
    accumulated then bias-added and stored immediately (staggered).
  - host gather: concat over (batch, token quarter).
"""

import numpy as np
import ml_dtypes

B, S, D = 2, 2048, 1024
H, HD = 16, 64
N_CORES = 8
G = 4                      # cores per batch group
HPC = 4                    # heads per core
CW = HPC * HD              # per-core projection width = 256
ATT_SCALE = float(HD) ** -0.5
P = 128
KC = D // P                # 8 contraction chunks
SC = S // P                # 16 key chunks of 128
NQ = 512                   # query chunk (= token quarter)
SQC = S // NQ              # 4 query chunks

LOG2E = 1.4426950408889634
# Schraudolph exp via int16 bitcast to bf16: bits = round(A16*x + B16)
A16 = 128.0 * LOG2E * ATT_SCALE
B16 = 128.0 * 127 - 7.33

# round-m 2-rank pairings (g <-> g^m) within each batch group
RG = {
    1: [[0, 1], [2, 3], [4, 5], [6, 7]],
    2: [[0, 2], [1, 3], [4, 6], [5, 7]],
    3: [[0, 3], [1, 2], [4, 7], [5, 6]],
}

_CACHED_NC = None


def _build():
    import concourse.mybir as mybir
    import concourse.tile as tile
    from concourse import bacc

    f32 = mybir.dt.float32
    bf16 = mybir.dt.bfloat16
    i16 = mybir.dt.int16
    Exp = mybir.ActivationFunctionType.Exp
    Ident = mybir.ActivationFunctionType.Identity
    add = mybir.AluOpType.add
    mult = mybir.AluOpType.mult

    nc = bacc.Bacc("TRN2", target_bir_lowering=False, debug=False,
                   num_devices=N_CORES)

    xqT = nc.declare_dram_parameter("xqT", [D, S], bf16, isOutput=False)
    wq = nc.declare_dram_parameter("wq", [D, CW], bf16, isOutput=False)
    wk = nc.declare_dram_parameter("wk", [D, CW], bf16, isOutput=False)
    wv = nc.declare_dram_parameter("wv", [D, CW], bf16, isOutput=False)
    bq = nc.declare_dram_parameter("bq", [CW], f32, isOutput=False)
    bk = nc.declare_dram_parameter("bk", [CW], f32, isOutput=False)
    bv = nc.declare_dram_parameter("bv", [CW], f32, isOutput=False)
    # wo pre-arranged per core: [ki=(l,hd), chunk=2(m-1)+j2, n] (m=1..3)
    wo = nc.declare_dram_parameter("wo", [P, 6, D], bf16, isOutput=False)
    # local-head wo rows, hd-stacked: [ki=(l,hd), j2, n]
    wolc = nc.declare_dram_parameter("wolc", [P, 2, D], bf16, isOutput=False)
    bo = nc.declare_dram_parameter("bo", [D], f32, isOutput=False)
    sel = nc.declare_dram_parameter("sel", [P, 3, 2], f32, isOutput=False)
    out = nc.declare_dram_parameter("out", [NQ, D], bf16, isOutput=True)

    with tile.TileContext(nc) as tc:
        with (
            tc.tile_pool(name="const", bufs=1) as const,
            tc.tile_pool(name="acts", bufs=1) as acts,
            tc.tile_pool(name="exps", bufs=8) as exps,
            tc.tile_pool(name="small", bufs=4) as small,
            tc.tile_pool(name="ostage", bufs=4) as ostage,
            tc.tile_pool(name="dram", bufs=1, space="DRAM") as dram,
        ):
            # ---- constant + activation loads (needed-first order) ----
            wq_sb = const.tile([P, KC, CW], bf16, tag="wq")
            wk_sb = const.tile([P, KC, CW], bf16, tag="wk")
            wv_sb = const.tile([P, KC, CW], bf16, tag="wv")
            bq_sb = const.tile([P, 2], f32, tag="bq")
            bk_sb = const.tile([P, 2], f32, tag="bk")
            nc.sync.dma_start(wq_sb[:], wq.ap().rearrange("(k p) m -> p k m", p=P))
            nc.sync.dma_start(bq_sb[:], bq.ap().rearrange("(j p) -> p j", p=P))
            # x quarters: separate tiles so compute can start on quarter 0
            xq_sb = [const.tile([P, KC, NQ], bf16, tag=f"xq{q}",
                                name=f"xq{q}") for q in range(SQC)]
            # ki-pair DMAs: big enough that the fixed HWDGE descriptor
            # cost stays under the transfer time, small enough to pace
            # the first projection chains.  wk is ordered after quarter 0
            # (the first q-projection chain buys it ~1.7us of cover).
            for q in range(SQC):
                for ki in range(0, KC, 2):
                    nc.sync.dma_start(
                        xq_sb[q][:, ki:ki + 2, :],
                        xqT[ki * P:(ki + 2) * P, q * NQ:(q + 1) * NQ]
                        .rearrange("(k p) m -> p k m", p=P))
                if q == 0:
                    nc.sync.dma_start(
                        wk_sb[:], wk.ap().rearrange("(k p) m -> p k m", p=P))
                    nc.sync.dma_start(
                        bk_sb[:], bk.ap().rearrange("(j p) -> p j", p=P))
            nc.sync.dma_start(wv_sb[:], wv.ap().rearrange("(k p) m -> p k m", p=P))
            bv_bc = const.tile([P, CW], f32, tag="bv")
            nc.sync.dma_start(bv_bc[:], bv.ap().partition_broadcast(P))
            # O-proj constants are not needed until ~the last third of
            # the kernel: stamp their loads past the x/weight DMAs so they
            # don't steal startup bandwidth
            sel_sb = const.tile([P, 3, 2], f32, tag="sel")
            wo_sb = const.tile([P, 6, D], bf16, tag="wo")
            wolc_sb = const.tile([P, 2, D], bf16, tag="wolc")
            bo_bc = const.tile([P, D], f32, tag="bo")
            with tc.tile_wait_until(0.025):
                nc.sync.dma_start(sel_sb[:], sel.ap())
                nc.sync.dma_start(wo_sb[:], wo.ap())
                nc.sync.dma_start(wolc_sb[:], wolc.ap())
                nc.sync.dma_start(bo_bc[:], bo.ap().partition_broadcast(P))

            qT_sb = acts.tile([P, 2, S], bf16, tag="qT")
            kT_sb = acts.tile([P, 2, S], bf16, tag="kT")
            # v per (mi, head): cols [v(64) | ones]
            v_sb = acts.tile([P, SC, HPC, HD + 1], bf16, tag="v")
            nc.vector.memset(v_sb[:, :, :, HD:HD + 1], 1.0)
            # local att, hd-stacked [l*64+hd, tok] per j2
            at_loc = [acts.tile([P, NQ], bf16, tag=f"atl{j2}",
                                name=f"atl{j2}") for j2 in range(2)]

            # staging/gather DRAM: stg_m [j2, P, NQ]; ago_m [rank, j2, P, NQ]
            stg = {m: dram.tile([2, P, NQ], bf16, tag=f"stg{m}",
                                name=f"stg{m}") for m in (1, 2, 3)}
            ago = {m: dram.tile([2, 2, P, NQ], bf16, tag=f"ago{m}",
                                name=f"ago{m}") for m in (1, 2, 3)}

            ucount = [0]
            with (
                tc.tile_pool(name="sc", bufs=4, space="PSUM") as scp,
                tc.tile_pool(name="pv", bufs=4, space="PSUM") as pvp,
            ):
                def qk_proj(j):
                    for si in range(SQC):
                        for (w_sb, b_sb, dst) in ((wq_sb, bq_sb, qT_sb),
                                                  (wk_sb, bk_sb, kT_sb)):
                            ps = scp.tile([P, NQ], f32, tag="sc",
                                          name=f"pp{j}_{si}_{dst is kT_sb}")
                            for ki in range(KC):
                                nc.tensor.matmul(
                                    ps,
                                    w_sb[:, ki, j * P:(j + 1) * P],
                                    xq_sb[si][:, ki, :],
                                    start=(ki == 0), stop=(ki == KC - 1),
                                )
                            # bias add on ACT (per-partition bias AP)
                            nc.scalar.activation(
                                dst[:, j, si * NQ:(si + 1) * NQ], ps,
                                Ident, bias=b_sb[:, j:j + 1], scale=1.0)

                def v_proj():
                    for si in range(SC):
                        pst = scp.tile([P, NQ], f32, tag="sc",
                                       name=f"ppv{si}")
                        ps = pst[:, :CW]
                        c = (si % 4) * P
                        for ki in range(KC):
                            nc.tensor.matmul(
                                ps,
                                xq_sb[si // 4][:, ki, c:c + P],
                                wv_sb[:, ki, :],
                                start=(ki == 0), stop=(ki == KC - 1),
                            )
                        with nc.allow_low_precision("bf16 v"):
                            nc.vector.tensor_tensor(
                                v_sb[:, si, :, :HD],
                                ps.rearrange("p (h x) -> p h x", x=HD),
                                bv_bc.rearrange("p (h x) -> p h x", x=HD),
                                add)

                def att_round(j2, sq, pending=None):
                    pvs = [pvp.tile([HD + 1, NQ], f32, tag="pv",
                                    name=f"pv{j2}_{sq}_{l}")
                           for l in range(2)]
                    for mi in range(SC):
                        if mi == 3 and pending is not None:
                            # previous round's softmax divide, interleaved
                            # here so it overlaps this round's exps
                            pending()
                            pending = None
                        # QK row-packed: head l at rows 64*l; per-head
                        # sct/et tiles keep the pipeline fine-grained
                        ets = []
                        for l in range(2):
                            o = l * HD
                            sct = scp.tile([P, NQ], f32, tag="sc",
                                           name=f"sc{j2}_{sq}_{mi}_{l}")
                            nc.tensor.matmul(
                                sct[:],
                                kT_sb[o:o + HD, j2, mi * P:(mi + 1) * P],
                                qT_sb[o:o + HD, j2, sq * NQ:(sq + 1) * NQ],
                                start=True, stop=True,
                            )
                            # exp: 3/5 ACT (exact), 2/5 DVE Schraudolph
                            use_act = (ucount[0] % 5) not in (1, 3)
                            ucount[0] += 1
                            et = exps.tile([P, NQ], bf16, tag="et")
                            if use_act:
                                nc.scalar.activation(
                                    et[:], sct[:], Exp, scale=ATT_SCALE)
                            else:
                                with nc.allow_low_precision(
                                        "schraudolph exp"):
                                    nc.vector.tensor_scalar(
                                        et[:].bitcast(i16), sct[:],
                                        A16, B16, mult, add)
                            ets.append(et)
                        for l in range(2):
                            h = 2 * j2 + l
                            nc.tensor.matmul(
                                pvs[l][:],
                                v_sb[:, mi, h, :],
                                ets[l][:],
                                start=(mi == 0), stop=(mi == SC - 1),
                            )
                    def divide():
                        # softmax divide: den copy off PSUM row 64 ->
                        # reciprocal -> gpsimd broadcast -> one multiply;
                        # stage for exchange (fire the round-m AllGather
                        # once the second j2 plane is staged).
                        rbs = []
                        for l in range(2):
                            # den move on ACT (handles the partition 64->0
                            # shift; the custom-ISA recip/broadcast only
                            # operate on a tile's partition 0)
                            den = small.tile([1, NQ], f32, tag="den",
                                             name=f"den{j2}{sq}{l}")
                            rec = small.tile([1, NQ], f32, tag="rec",
                                             name=f"rec{j2}{sq}{l}")
                            rb = small.tile([HD, NQ], f32, tag="rb",
                                            name=f"rb{j2}{sq}{l}")
                            nc.scalar.activation(
                                den[:], pvs[l][HD:HD + 1, :], Ident,
                                scale=1.0)
                            with nc.allow_low_precision("approx recip"):
                                nc.vector.reciprocal_approx_fast(
                                    rec[:], den[:])
                            nc.gpsimd.partition_broadcast(rb[:], rec[:],
                                                          channels=HD)
                            rbs.append(rb)
                        # local rounds: l=1 first so its partition-shift
                        # DMA overlaps l=0's multiply (which needs none)
                        for l in ((1, 0) if sq == 0 else (0, 1)):
                            if sq == 0 and l == 0:
                                at = at_loc[j2][0:HD, :]
                            else:
                                att = small.tile([HD, NQ], bf16, tag="at",
                                                 name=f"at{j2}{sq}{l}")
                                at = att[:]
                            with nc.allow_low_precision("bf16 att"):
                                nc.vector.tensor_tensor(
                                    at, pvs[l][:HD, :], rbs[l][:], mult)
                            if sq == 0:
                                if l == 1:
                                    nc.sync.dma_start(at_loc[j2][HD:P, :],
                                                      at)
                            else:
                                nc.sync.dma_start(
                                    stg[sq][j2, l * HD:(l + 1) * HD, :], at)
                        if sq != 0 and j2 == 1:
                            nc.gpsimd.collective_compute(
                                "AllGather", mybir.AluOpType.bypass,
                                replica_groups=RG[sq],
                                ins=[stg[sq][:, :, :]],
                                outs=[ago[sq].opt()],
                            )

                    return divide

                qk_proj(0)
                v_proj()
                qk_proj(1)
                pend = None
                for j2, sq in ((0, 1), (1, 1), (0, 2), (1, 2), (0, 3),
                               (1, 3), (0, 0), (1, 0)):
                    pend = att_round(j2, sq, pending=pend)
                pend()

                # ---- O-projection (token-sharded, full width) -------
                # po tiles reuse the sc PSUM slots so the first local
                # matmuls start while the last round's divide drains (a
                # fresh pool would wait for ALL attention PSUM).
                # Blends are wait-stamped past the end of the rounds:
                # without it the scheduler parks an engine on them the
                # moment the collective lands mid-round, starving PE
                # (runtime order is stream position + semaphores, so the
                # stamp costs nothing).
                atk = {}
                for m in (1, 2, 3):
                    for j2 in range(2):
                        a2 = acts.tile([P, 2, NQ], bf16, tag=f"a2_{j2}{m}",
                                       name=f"a2_{j2}_{m}")
                        # stamped so the load sits in the DMA queues behind
                        # the rounds' stg/shift writes (else those get
                        # stuck behind it and stall round ends); m=1,2
                        # early enough that their blends can run in DVE
                        # slack during the local rounds
                        with tc.tile_wait_until(0.155 if m < 3 else 0.174):
                            nc.sync.dma_start(
                                a2[:],
                                ago[m][:, j2, :, :].rearrange("r h s -> h r s"))
                        t0 = small.tile([P, NQ], bf16, tag="t0",
                                        name=f"t0_{j2}_{m}")
                        ak = acts.tile([P, NQ], bf16, tag=f"atk{j2}{m}",
                                       name=f"atk{j2}_{m}")
                        with (tc.tile_wait_until({1: 0.16, 2: 0.182,
                                                  3: 0.185}[m]),
                              nc.allow_low_precision("half select")):
                            nc.vector.tensor_scalar(
                                t0[:], a2[:, 0, :],
                                sel_sb[:, m - 1, 0:1], None, mult)
                            nc.vector.scalar_tensor_tensor(
                                ak[:], a2[:, 1, :],
                                sel_sb[:, m - 1, 1:2], t0[:], mult, add)
                        atk[(j2, m)] = ak

                for tc_i in range(SQC):
                    for ch in range(2):
                        # first half of the tiles on the sc slots, second
                        # half on the pv banks (idle after the last
                        # divide) so all 8 chains pipeline freely
                        pool, tg = ((scp, "sc") if tc_i < 2 else
                                    (pvp, "pv"))
                        po = pool.tile([P, NQ], f32, tag=tg,
                                       name=f"po{tc_i}_{ch}")
                        nsl = ch * NQ
                        srcs = [(at_loc[0], wolc_sb[:, 0, nsl:nsl + NQ])]
                        srcs += [(atk[(j2, m)], wo_sb[:, 2 * (m - 1) + j2,
                                                      nsl:nsl + NQ])
                                for m in (1, 2) for j2 in range(2)]
                        srcs += [(at_loc[1], wolc_sb[:, 1, nsl:nsl + NQ])]
                        srcs += [(atk[(j2, 3)], wo_sb[:, 4 + j2,
                                                      nsl:nsl + NQ])
                                for j2 in range(2)]
                        for idx, (a, w) in enumerate(srcs):
                            nc.tensor.matmul(
                                po[:], a[:, tc_i * P:(tc_i + 1) * P], w,
                                start=(idx == 0), stop=(idx == len(srcs) - 1),
                            )
                        # bf16 staging halves the tail store bandwidth
                        ot = ostage.tile([P, NQ], bf16, tag="ot")
                        with nc.allow_low_precision("bf16 out"):
                            nc.vector.tensor_tensor(
                                ot[:], po[:], bo_bc[:, nsl:nsl + NQ], add)
                        nc.sync.dma_start(
                            out[tc_i * P:(tc_i + 1) * P, nsl:nsl + NQ],
                            ot[:])

    nc.compile()
    return nc


def _get_nc():
    global _CACHED_NC
    if _CACHED_NC is None:
        _CACHED_NC = _build()
    return _CACHED_NC


def _arrange_wo(wo_np, g):
    """wo [D, D] -> per-core ([ki=(l,hd), chunk=2(m-1)+j2, n] for the
    exchanged rounds m=1..3, plus hd-stacked [ki=(l,hd), j2, n] for the
    local heads).

    Exchanged chunk 2(m-1)+j2 holds rows for heads 4*(g^m) + 2*j2 + l,
    l stacked on ki (rows l*64+hd).  Local chunk j2: heads 4g+2*j2+l."""
    wo_r = wo_np.reshape(H, HD, D)
    arr = np.empty((P, 6, D), dtype=np.float32)
    for m in (1, 2, 3):
        for j2 in range(2):
            for l in range(2):
                head = 4 * (g ^ m) + 2 * j2 + l
                arr[l * HD:(l + 1) * HD, 2 * (m - 1) + j2, :] = wo_r[head]
    loc = np.empty((P, 2, D), dtype=np.float32)
    for j2 in range(2):
        for l in range(2):
            loc[l * HD:(l + 1) * HD, j2, :] = wo_r[4 * g + 2 * j2 + l]
    return arr, loc


def kernel(x, wq, bq, wk, bk, wv, bv, wo, bo):
    from concourse.bass_utils import run_bass_kernel_spmd

    x = np.asarray(x, dtype=np.float32)
    wq = np.asarray(wq, dtype=np.float32)
    wk = np.asarray(wk, dtype=np.float32)
    wv = np.asarray(wv, dtype=np.float32)
    wo = np.asarray(wo, dtype=np.float32)
    bq = np.asarray(bq, dtype=np.float32)
    bk = np.asarray(bk, dtype=np.float32)
    bv = np.asarray(bv, dtype=np.float32)
    bo = np.asarray(bo, dtype=np.float32)

    nc = _get_nc()

    bf = ml_dtypes.bfloat16
    in_maps = []
    for c in range(N_CORES):
        b, g = c // G, c % G
        cs = slice(g * CW, (g + 1) * CW)
        xt = np.ascontiguousarray(x[b].T)
        xq = np.concatenate(
            [xt[:, (s ^ g) * NQ:((s ^ g) + 1) * NQ] for s in range(SQC)],
            axis=1)
        sel = np.zeros((P, 3, 2), dtype=np.float32)
        for m in (1, 2, 3):
            low = (g ^ m) < g
            sel[:, m - 1, 0 if low else 1] = 1.0
        wo_a, wolc_a = _arrange_wo(wo, g)
        in_maps.append({
            "xqT": np.ascontiguousarray(xq).astype(bf),
            "wq": np.ascontiguousarray(wq[:, cs]).astype(bf),
            "wk": np.ascontiguousarray(wk[:, cs]).astype(bf),
            "wv": np.ascontiguousarray(wv[:, cs]).astype(bf),
            "bq": np.ascontiguousarray(bq[cs]),
            "bk": np.ascontiguousarray(bk[cs]),
            "bv": np.ascontiguousarray(bv[cs]),
            "wo": wo_a.astype(bf),
            "wolc": wolc_a.astype(bf),
            "bo": bo,
            "sel": sel,
        })

    res = run_bass_kernel_spmd(nc, in_maps, core_ids=list(range(N_CORES)))

    full = np.empty((B, S, D), dtype=np.float32)
    for c in range(N_CORES):
        b, g = c // G, c % G
        full[b, g * NQ:(g + 1) * NQ, :] = np.asarray(
            res.results[c]["out"], dtype=np.float32)
    return full
